# revision 1
# baseline (speedup 1.0000x reference)
"""FM (DeepFM first+second order) multi-task kernel for Trainium2, 8 NeuronCores.

Strategy: data-parallel over batch (2048 rows/core), embedding tables
replicated on every core as ONE combined table [F*V, 65] f32 whose rows pack
emb2[f,v,:64] in cols 0:64 and emb1[f,v,0] in col 64, so one 260B descriptor
fetches both tables' entries for a (batch, field) pair.  Gathers use the
gpsimd indirect DMA in its hardware-validated form: one int32 index per
partition, each partition fetching one table row (out [128, 65] per field).
Per 128-batch tile: 26 such gathers (partition = batch element), then
S = sum_f rows as one strided DVE reduce, the FM sum-of-squares term as one
ACT Square+accumulate pass over the gathered block, and the dense linear /
sigmoid heads as tiny DVE/ACT ops.  No collectives.  The kernel is
descriptor-generation bound on gpsimd (~1.46us per 128-row gather, 416
gathers/core ~= 610us), ~95x faster than the single-core jax reference.
"""

import os
import sys

import numpy as np

if "/opt/trn_rl_repo" not in sys.path:
    sys.path.insert(0, "/opt/trn_rl_repo")

N_DENSE = 13
F = 26           # n sparse fields
V = 100000       # vocab
E = 64           # emb dim
B = 16384        # global batch
N_CORES = 8
BC = B // N_CORES        # 2048 batch rows per core
TB = 128                 # batch tile (= SBUF partitions)
NT = BC // TB            # 16 tiles per core
ROW = 65                 # combined-table row width (f32): 64 emb2 + 1 emb1, no pad
AUXW = 19                # aux scalar vector width

_NC_CACHE = {}


def build_nc(debug=False):
    import concourse.bass as bass
    import concourse.tile as tile
    from concourse import bacc, mybir
    from contextlib import ExitStack

    f32 = mybir.dt.float32
    i32 = mybir.dt.int32
    Square = mybir.ActivationFunctionType.Square
    Sigmoid = mybir.ActivationFunctionType.Sigmoid
    add = mybir.AluOpType.add
    mult = mybir.AluOpType.mult

    nc = bacc.Bacc(
        "TRN2", target_bir_lowering=False, debug=debug, num_devices=N_CORES
    )

    table = nc.dram_tensor("table", [F * V, ROW], f32, kind="ExternalInput").ap()
    idxs = nc.dram_tensor("idxs", [NT, TB, F], i32, kind="ExternalInput").ap()
    dense = nc.dram_tensor("dense", [NT, TB, N_DENSE], f32, kind="ExternalInput").ap()
    aux = nc.dram_tensor("aux", [TB, AUXW], f32, kind="ExternalInput").ap()
    fin = nc.dram_tensor("finish", [NT, TB, 1], f32, kind="ExternalOutput").ap()
    lik = nc.dram_tensor("like", [NT, TB, 1], f32, kind="ExternalOutput").ap()

    sqrt_half = float(np.sqrt(0.5, dtype=np.float64))

    with tile.TileContext(nc) as tc, ExitStack() as ctx:
        singles = ctx.enter_context(tc.tile_pool(name="singles", bufs=1))
        gpool = ctx.enter_context(tc.tile_pool(name="g", bufs=6))
        inpool = ctx.enter_context(tc.tile_pool(name="inp", bufs=4))
        sqpool = ctx.enter_context(tc.tile_pool(name="sq", bufs=2))
        spool = ctx.enter_context(tc.tile_pool(name="s", bufs=4))
        outpool = ctx.enter_context(tc.tile_pool(name="o", bufs=4))

        aux_t = singles.tile([TB, AUXW], f32)
        nc.sync.dma_start(out=aux_t[:], in_=aux[:])

        for t in range(NT):
            idx_t = inpool.tile([TB, F], i32)
            nc.sync.dma_start(out=idx_t[:], in_=idxs[t])
            d_t = inpool.tile([TB, N_DENSE], f32)
            nc.sync.dma_start(out=d_t[:], in_=dense[t])

            # Gather 26 combined rows per batch element: g_t[p, f, :] =
            # table[idx_t[p, f], :].  The HW indirect DMA pairs one index per
            # partition and fetches out's free size contiguously, so issue one
            # gather per field writing a [128, 128] column slice.
            g_t = gpool.tile([TB, F, ROW], f32)
            for f in range(F):
                nc.gpsimd.indirect_dma_start(
                    out=g_t[:, f, :],
                    out_offset=None,
                    in_=table[:],
                    in_offset=bass.IndirectOffsetOnAxis(
                        ap=idx_t[:, f : f + 1], axis=0
                    ),
                )

            # S_ext[p, e] = sum_f g[p, f, e] for e in 0..64 (col 64 = emb1 sum)
            s_t = spool.tile([TB, E + 1], f32)
            nc.vector.tensor_reduce(
                out=s_t[:],
                in_=g_t[:, :, 0 : E + 1].rearrange("p f e -> p e f"),
                axis=mybir.AxisListType.X,
                op=add,
            )

            # qs[p] = 0.5 * sum_{f,e} g^2  (scale inside Square pre-halves)
            sq_t = sqpool.tile([TB, F, E], f32)
            qs_t = spool.tile([TB, 1], f32)
            nc.scalar.activation(
                out=sq_t[:],
                in_=g_t[:, :, 0:E],
                func=Square,
                scale=sqrt_half,
                bias=aux_t[:, 18:19],  # 0.0
                accum_out=qs_t[:],
            )

            # ss[p] = 0.5 * sum_e S^2  (same validated ACT Square+accum form)
            s2_t = sqpool.tile([TB, E], f32)
            ss_t = spool.tile([TB, 1], f32)
            nc.scalar.activation(
                out=s2_t[:],
                in_=s_t[:, 0:E],
                func=Square,
                scale=sqrt_half,
                bias=aux_t[:, 18:19],  # 0.0
                accum_out=ss_t[:],
            )

            # do[p] = sum_k dense[p,k] * W_dense[k]   (+ b_dense added below)
            dsc_t = sqpool.tile([TB, N_DENSE], f32)
            do_t = spool.tile([TB, 1], f32)
            nc.vector.tensor_mul(dsc_t[:], d_t[:], aux_t[:, 0:N_DENSE])
            nc.vector.tensor_reduce(
                out=do_t[:], in_=dsc_t[:], axis=mybir.AxisListType.X, op=add
            )

            # logits = (do + b_dense) + lin_sparse_sum + (ss - qs)
            df_t = spool.tile([TB, 1], f32)
            l1_t = spool.tile([TB, 1], f32)
            l2_t = spool.tile([TB, 1], f32)
            lg_t = spool.tile([TB, 1], f32)
            nc.vector.tensor_sub(df_t[:], ss_t[:], qs_t[:])
            nc.vector.tensor_add(l1_t[:], do_t[:], aux_t[:, 13:14])
            nc.vector.tensor_add(l2_t[:], l1_t[:], s_t[:, E : E + 1])
            nc.vector.tensor_add(lg_t[:], l2_t[:], df_t[:])

            fin_t = outpool.tile([TB, 1], f32)
            lik_t = outpool.tile([TB, 1], f32)
            nc.scalar.activation(
                out=fin_t[:], in_=lg_t[:], func=Sigmoid,
                scale=aux_t[:, 14:15], bias=aux_t[:, 15:16],
            )
            nc.scalar.activation(
                out=lik_t[:], in_=lg_t[:], func=Sigmoid,
                scale=aux_t[:, 16:17], bias=aux_t[:, 17:18],
            )
            nc.sync.dma_start(out=fin[t], in_=fin_t[:])
            nc.sync.dma_start(out=lik[t], in_=lik_t[:])

    nc.compile()
    return nc


def _get_nc():
    if "nc" not in _NC_CACHE:
        _NC_CACHE["nc"] = build_nc(debug=False)
    return _NC_CACHE["nc"]


def _prepare_inputs(sparse_inputs, dense_inputs, emb1, emb2, W_dense, b_dense,
                    W_finish, b_finish, W_like, b_like):
    sparse_inputs = np.asarray(sparse_inputs)
    dense_inputs = np.asarray(dense_inputs, dtype=np.float32)
    emb1 = np.asarray(emb1, dtype=np.float32)
    emb2 = np.asarray(emb2, dtype=np.float32)

    T = np.zeros((F * V, ROW), dtype=np.float32)
    T[:, :E] = emb2.reshape(F * V, E)
    T[:, E] = emb1.reshape(F * V)

    aux = np.zeros((TB, AUXW), dtype=np.float32)
    aux[:, 0:N_DENSE] = np.asarray(W_dense, dtype=np.float32).reshape(-1)
    aux[:, 13] = np.float32(np.asarray(b_dense).reshape(-1)[0])
    aux[:, 14] = np.float32(np.asarray(W_finish).reshape(-1)[0])
    aux[:, 15] = np.float32(np.asarray(b_finish).reshape(-1)[0])
    aux[:, 16] = np.float32(np.asarray(W_like).reshape(-1)[0])
    aux[:, 17] = np.float32(np.asarray(b_like).reshape(-1)[0])

    field_off = (np.arange(F, dtype=np.int64) * V)[None, :]
    flat = (sparse_inputs.astype(np.int64) + field_off).astype(np.int32)  # [B, F]

    in_maps = []
    for c in range(N_CORES):
        sl = slice(c * BC, (c + 1) * BC)
        in_maps.append(dict(
            table=T,
            idxs=np.ascontiguousarray(flat[sl].reshape(NT, TB, F)),
            dense=np.ascontiguousarray(dense_inputs[sl].reshape(NT, TB, N_DENSE)),
            aux=aux,
        ))
    return in_maps


def _install_trace_hooks():
    """Make trace=True work in containers whose antenv stub lacks axon_hooks.

    Injects an antenv.axon_hooks module backed by the libaxon_pjrt ctypes NRT
    profile hook, and stubs out the artifact upload (no bucket access here).
    """
    import sys
    import types

    try:
        from antenv.axon_hooks import get_axon_ntff_profile_hook  # noqa: F401
    except ImportError:
        mod = types.ModuleType("antenv.axon_hooks")
        mod._hook = None
        mod.set_axon_ntff_profile_hook = lambda h: setattr(mod, "_hook", h)
        mod.get_axon_ntff_profile_hook = lambda: mod._hook
        sys.modules["antenv.axon_hooks"] = mod
        import antenv

        antenv.axon_hooks = mod
        from trn_agent_boot.trn_boot import _ntff_profile_via_ctypes

        mod._hook = _ntff_profile_via_ctypes("/opt/axon/libaxon_pjrt.so")

    from concourse import bass_utils

    bass_utils.upload_artifacts = lambda tmpdir: f"local://{tmpdir}"


def run(inputs, trace=False, cores=None):
    """Run on the NeuronCores; returns ((finish, like), BassKernelResults)."""
    from concourse.bass_utils import run_bass_kernel_spmd

    if trace:
        _install_trace_hooks()
    in_maps = _prepare_inputs(**inputs)
    nc = _get_nc()
    ncores = cores if cores is not None else N_CORES
    res = run_bass_kernel_spmd(nc, in_maps[:ncores], list(range(ncores)), trace=trace)
    fin = np.concatenate(
        [res.results[c]["finish"].reshape(BC, 1) for c in range(ncores)], axis=0
    )
    lik = np.concatenate(
        [res.results[c]["like"].reshape(BC, 1) for c in range(ncores)], axis=0
    )
    return (fin, lik), res


def kernel(**inputs):
    (fin, lik), _ = run(inputs, trace=bool(int(os.environ.get("KERNEL_TRACE", "0"))))
    return fin, lik



# revision 7
# speedup vs baseline: 1.2393x; 1.2393x over previous
"""FM (DeepFM first+second order) multi-task kernel for Trainium2, 8 NeuronCores.

Strategy: data-parallel over batch (2048 rows/core).  The gather uses the
dedicated SWDGE dma_gather primitive (vectorized Q7 descriptor generation,
~30x cheaper per descriptor than generic indirect DMA) with int16 indices.
To fit the 100k vocab into int16 range, two vocab rows are packed per 512-byte
table slot (slot id = v>>1 in [0, 50000)) and the call's base pointer sits at
the field's slot midpoint so signed indices (v>>1)-25000 in [-25000, 25000)
cover the whole field.  Each gathered slot holds both candidate rows
interleaved (elem[2e+g] = emb2[2s+g, e], fp16) plus both emb1 scalars; a
host-precomputed one-hot (v&1) mask selects the right candidate on-chip:
DVE mask-multiply + unit-stride pair reduce, ACT squares, tiny finalization.
26 dma_gather calls per core (one per field, 2176 idxs each incl. pad chunk
that keeps the trailing index non-negative so the ucode's trailing-trim
doesn't drop real rows).
"""

import os
import sys

import numpy as np

if "/opt/trn_rl_repo" not in sys.path:
    sys.path.insert(0, "/opt/trn_rl_repo")

N_DENSE = 13
F = 26           # n sparse fields
V = 100000       # vocab
E = 64           # emb dim
B = 16384        # global batch
N_CORES = 8
BC = B // N_CORES        # 2048 batch rows per core
TB = 128                 # SBUF partitions
NC_ = BC // TB           # 16 sample chunks per core (out free dim)
G = 2                    # vocab rows packed per slot
S_FIELD = V // G         # 50000 slots per field
MID = S_FIELD // 2       # 25000: signed-index midpoint
ROWE = 256               # fp16 elems per slot (512 B)
NIDX = BC + TB           # 2176: padded index count (17 chunks of 128)
NCHUNK = NIDX // TB      # 17
AUXW = 19                # aux scalar vector width

_NC_CACHE = {}


def build_nc(debug=False):
    import concourse.bass as bass
    import concourse.tile as tile
    from concourse import bacc, mybir
    from contextlib import ExitStack

    f32 = mybir.dt.float32
    f16 = mybir.dt.float16
    i16 = mybir.dt.int16
    Square = mybir.ActivationFunctionType.Square
    Sigmoid = mybir.ActivationFunctionType.Sigmoid
    add = mybir.AluOpType.add

    nc = bacc.Bacc(
        "TRN2", target_bir_lowering=False, debug=debug, num_devices=N_CORES
    )

    table = nc.dram_tensor("table", [F * S_FIELD, ROWE], f16, kind="ExternalInput").ap()
    idxs = nc.dram_tensor("idxs", [TB, F, NIDX // 16], i16, kind="ExternalInput").ap()
    masks = nc.dram_tensor("masks", [TB, F, NC_, G], f16, kind="ExternalInput").ap()
    dense = nc.dram_tensor("dense", [NC_, TB, N_DENSE], f32, kind="ExternalInput").ap()
    aux = nc.dram_tensor("aux", [TB, AUXW], f32, kind="ExternalInput").ap()
    fin = nc.dram_tensor("finish", [NC_, TB, 1], f32, kind="ExternalOutput").ap()
    lik = nc.dram_tensor("like", [NC_, TB, 1], f32, kind="ExternalOutput").ap()

    with tile.TileContext(nc) as tc, ExitStack() as ctx:
        ctx.enter_context(
            nc.allow_low_precision(
                reason="fp16 pair-select/field sums; 2e-2 output tolerance"
            )
        )
        singles = ctx.enter_context(tc.tile_pool(name="singles", bufs=1))
        gpool = ctx.enter_context(tc.tile_pool(name="g", bufs=4))
        wpool = ctx.enter_context(tc.tile_pool(name="w", bufs=3))
        opool = ctx.enter_context(tc.tile_pool(name="o", bufs=2))

        aux_t = singles.tile([TB, AUXW], f32)
        nc.sync.dma_start(out=aux_t[:], in_=aux[:])
        idx_t = singles.tile([TB, F, NIDX // 16], i16)
        nc.sync.dma_start(out=idx_t[:], in_=idxs[:])
        mask_t = singles.tile([TB, F, NC_, G], f16)
        nc.sync.dma_start(out=mask_t[:], in_=masks[:])
        d_t = singles.tile([TB, NC_, N_DENSE], f32)
        nc.sync.dma_start(out=d_t[:], in_=dense[:].rearrange("t p k -> p t k"))

        # accumulators over fields
        S_t = singles.tile([TB, NC_, E], f16)     # sum of selected emb2 rows
        Q_t = singles.tile([TB, NC_, E], f16)     # sum of squared selected rows
        S1_t = singles.tile([TB, NC_], f16)       # sum of selected emb1 scalars
        nc.vector.memset(S_t[:], 0.0)
        nc.vector.memset(Q_t[:], 0.0)
        nc.vector.memset(S1_t[:], 0.0)

        for f in range(F):
            g_t = gpool.tile([TB, NCHUNK, ROWE], f16)
            nc.gpsimd.dma_gather(
                g_t[:],
                table[f * S_FIELD + MID :, :],
                idx_t[:, f, :],
                NIDX,
                NIDX,
                ROWE,
                single_packet=False,
            )
            # mask-select the right row of each slot pair (g innermost)
            mb = mask_t[:, f].unsqueeze(2).broadcast_to([TB, NC_, E, G])
            m1 = wpool.tile([TB, NC_, E, G], f16)
            g2 = g_t[:, 0:NC_, 0 : E * G].rearrange("p c (e g) -> p c e g", g=G)
            nc.vector.tensor_mul(m1[:], g2, mb)
            sel = wpool.tile([TB, NC_, E], f16)
            nc.vector.tensor_reduce(
                out=sel[:], in_=m1[:], axis=mybir.AxisListType.X, op=add
            )
            nc.vector.tensor_add(S_t[:], S_t[:], sel[:])
            sq = wpool.tile([TB, NC_, E], f16)
            nc.scalar.activation(
                out=sq[:], in_=sel[:], func=Square, scale=1.0,
            )
            nc.vector.tensor_add(Q_t[:], Q_t[:], sq[:])
            # emb1 select (elems [E*G, E*G+G) of each slot)
            m2 = wpool.tile([TB, NC_, G], f16)
            nc.vector.tensor_mul(
                m2[:], g_t[:, 0:NC_, E * G : E * G + G], mask_t[:, f]
            )
            s1 = wpool.tile([TB, NC_], f16)
            nc.vector.tensor_reduce(
                out=s1[:], in_=m2[:], axis=mybir.AxisListType.X, op=add
            )
            nc.vector.tensor_add(S1_t[:], S1_t[:], s1[:])

        # ---- finalize (per sample = (partition, chunk)) ----
        # ss = sum_e S^2, qs = sum_e Q
        s2 = wpool.tile([TB, NC_, E], f32)
        nc.vector.tensor_mul(s2[:], S_t[:], S_t[:])
        ss = opool.tile([TB, NC_], f32)
        nc.vector.tensor_reduce(out=ss[:], in_=s2[:], axis=mybir.AxisListType.X, op=add)
        qs = opool.tile([TB, NC_], f32)
        nc.vector.tensor_reduce(
            out=qs[:], in_=Q_t[:], axis=mybir.AxisListType.X, op=add
        )
        # dense linear: do = sum_k dense[p,c,k] * W[k]
        wb = aux_t[:, 0:N_DENSE].unsqueeze(1).broadcast_to([TB, NC_, N_DENSE])
        dsc = wpool.tile([TB, NC_, N_DENSE], f32)
        nc.vector.tensor_mul(dsc[:], d_t[:], wb)
        do = opool.tile([TB, NC_], f32)
        nc.vector.tensor_reduce(
            out=do[:], in_=dsc[:], axis=mybir.AxisListType.X, op=add
        )
        # logits = do + b_dense + S1 + 0.5*(ss - qs)
        df = opool.tile([TB, NC_], f32)
        nc.vector.tensor_sub(df[:], ss[:], qs[:])
        l1 = opool.tile([TB, NC_], f32)
        nc.vector.tensor_scalar_mul(l1[:], df[:], 0.5)
        l2 = opool.tile([TB, NC_], f32)
        nc.vector.tensor_add(l2[:], l1[:], do[:])
        l3 = opool.tile([TB, NC_], f32)
        nc.vector.tensor_add(l3[:], l2[:], S1_t[:])
        bb = aux_t[:, 13:14].broadcast_to([TB, NC_])
        lg = opool.tile([TB, NC_], f32)
        nc.vector.tensor_add(lg[:], l3[:], bb)

        fin_t = opool.tile([TB, NC_], f32)
        lik_t = opool.tile([TB, NC_], f32)
        nc.scalar.activation(
            out=fin_t[:], in_=lg[:], func=Sigmoid,
            scale=aux_t[:, 14:15], bias=aux_t[:, 15:16],
        )
        nc.scalar.activation(
            out=lik_t[:], in_=lg[:], func=Sigmoid,
            scale=aux_t[:, 16:17], bias=aux_t[:, 17:18],
        )
        nc.sync.dma_start(
            out=fin[:].rearrange("t p e -> p t e"), in_=fin_t[:].unsqueeze(2)
        )
        nc.sync.dma_start(
            out=lik[:].rearrange("t p e -> p t e"), in_=lik_t[:].unsqueeze(2)
        )

    nc.compile()
    return nc


def _get_nc():
    if "nc" not in _NC_CACHE:
        _NC_CACHE["nc"] = build_nc(debug=False)
    return _NC_CACHE["nc"]


def _prepare_inputs(sparse_inputs, dense_inputs, emb1, emb2, W_dense, b_dense,
                    W_finish, b_finish, W_like, b_like):
    sparse_inputs = np.asarray(sparse_inputs)
    dense_inputs = np.asarray(dense_inputs, dtype=np.float32)
    emb1 = np.asarray(emb1, dtype=np.float32)
    emb2 = np.asarray(emb2, dtype=np.float32)

    # slot table: [F*S_FIELD, 256] fp16; slot s of field f covers v in
    # {2s, 2s+1}: elems[2e+g] = emb2[f, 2s+g, e]; elems[128+g] = emb1[f, 2s+g]
    T = np.zeros((F * S_FIELD, ROWE), dtype=np.float16)
    e2 = emb2.astype(np.float16).reshape(F, S_FIELD, G, E).transpose(0, 1, 3, 2)
    T[:, 0 : E * G] = e2.reshape(F * S_FIELD, E * G)
    T[:, E * G : E * G + G] = (
        emb1.astype(np.float16).reshape(F, S_FIELD, G).reshape(F * S_FIELD, G)
    )

    aux = np.zeros((TB, AUXW), dtype=np.float32)
    aux[:, 0:N_DENSE] = np.asarray(W_dense, dtype=np.float32).reshape(-1)
    aux[:, 13] = np.float32(np.asarray(b_dense).reshape(-1)[0])
    aux[:, 14] = np.float32(np.asarray(W_finish).reshape(-1)[0])
    aux[:, 15] = np.float32(np.asarray(b_finish).reshape(-1)[0])
    aux[:, 16] = np.float32(np.asarray(W_like).reshape(-1)[0])
    aux[:, 17] = np.float32(np.asarray(b_like).reshape(-1)[0])

    v_all = sparse_inputs.astype(np.int64)          # [B, F] in [0, V)
    slot_all = (v_all >> 1) - MID                   # signed slot offsets
    gsel_all = (v_all & 1).astype(np.int64)         # which row of the pair

    in_maps = []
    j = np.arange(BC)
    ch16, col = (j % 16), (j // 16)
    for c in range(N_CORES):
        sl = slice(c * BC, (c + 1) * BC)
        slot = slot_all[sl]                         # [BC, F]
        gsel = gsel_all[sl]
        # idx16 [128, F, NIDX//16]: position j -> (partition j%16 (+16k), col j//16)
        idx16 = np.full((TB, F, NIDX // 16), 1000, dtype=np.int16)
        tmp = np.zeros((16, F, NIDX // 16), dtype=np.int16)
        tmp[:, :, : BC // 16] = 1000
        tmp[ch16, :, col] = slot.astype(np.int16)
        idx16[:] = np.tile(tmp, (8, 1, 1))
        # masks [128, F, NC_, G] one-hot of gsel; sample j = chunk*128 + p
        mk = np.zeros((TB, F, NC_, G), dtype=np.float16)
        p_of_j, c_of_j = (j % TB), (j // TB)
        for g in range(G):
            sel = (gsel == g)                       # [BC, F]
            mk[p_of_j[:, None], np.arange(F)[None, :], c_of_j[:, None], g] = (
                sel.astype(np.float16)
            )
        in_maps.append(dict(
            table=T,
            idxs=idx16,
            masks=mk,
            dense=np.ascontiguousarray(
                dense_inputs[sl].reshape(NC_, TB, N_DENSE)
            ),
            aux=aux,
        ))
    return in_maps


def _install_trace_hooks():
    """Make trace=True work in containers whose antenv stub lacks axon_hooks."""
    import sys
    import types

    try:
        from antenv.axon_hooks import get_axon_ntff_profile_hook  # noqa: F401
    except ImportError:
        mod = types.ModuleType("antenv.axon_hooks")
        mod._hook = None
        mod.set_axon_ntff_profile_hook = lambda h: setattr(mod, "_hook", h)
        mod.get_axon_ntff_profile_hook = lambda: mod._hook
        sys.modules["antenv.axon_hooks"] = mod
        import antenv

        antenv.axon_hooks = mod
        from trn_agent_boot.trn_boot import _ntff_profile_via_ctypes

        mod._hook = _ntff_profile_via_ctypes("/opt/axon/libaxon_pjrt.so")

    from concourse import bass_utils

    bass_utils.upload_artifacts = lambda tmpdir: f"local://{tmpdir}"


def run(inputs, trace=False, cores=None):
    """Run on the NeuronCores; returns ((finish, like), BassKernelResults)."""
    from concourse.bass_utils import run_bass_kernel_spmd

    if trace:
        _install_trace_hooks()
    in_maps = _prepare_inputs(**inputs)
    nc = _get_nc()
    ncores = cores if cores is not None else N_CORES
    res = run_bass_kernel_spmd(nc, in_maps[:ncores], list(range(ncores)), trace=trace)
    fin = np.concatenate(
        [res.results[c]["finish"].reshape(BC, 1) for c in range(ncores)], axis=0
    )
    lik = np.concatenate(
        [res.results[c]["like"].reshape(BC, 1) for c in range(ncores)], axis=0
    )
    return (fin, lik), res


def kernel(**inputs):
    (fin, lik), _ = run(inputs, trace=bool(int(os.environ.get("KERNEL_TRACE", "0"))))
    return fin, lik


# revision 10
# speedup vs baseline: 2.6638x; 2.1494x over previous
"""FM (DeepFM first+second order) multi-task kernel for Trainium2, 8 NeuronCores.

Strategy: data-parallel over batch (2048 rows/core).  The gather uses the
dedicated SWDGE dma_gather primitive (vectorized Q7 descriptor generation,
~30x cheaper per descriptor than generic indirect DMA) with int16 indices.
To fit the 100k vocab into int16 range, two vocab rows are packed per 512-byte
table slot (slot id = v>>1 in [0, 50000)) and the call's base pointer sits at
the field's slot midpoint so signed indices (v>>1)-25000 in [-25000, 25000)
cover the whole field.  Each gathered slot holds both candidate rows
interleaved (elem[2e+g] = emb2[2s+g, e], fp16) plus both emb1 scalars; a
host-precomputed one-hot (v&1) mask selects the right candidate on-chip:
DVE mask-multiply + unit-stride pair reduce, ACT squares, tiny finalization.
26 dma_gather calls per core (one per field, 2176 idxs each incl. pad chunk
that keeps the trailing index non-negative so the ucode's trailing-trim
doesn't drop real rows).
"""

import os
import sys

import numpy as np

if "/opt/trn_rl_repo" not in sys.path:
    sys.path.insert(0, "/opt/trn_rl_repo")

N_DENSE = 13
F = 26           # n sparse fields
V = 100000       # vocab
E = 64           # emb dim
B = 16384        # global batch
N_CORES = 8
BC = B // N_CORES        # 2048 batch rows per core
TB = 128                 # SBUF partitions
NC_ = BC // TB           # 16 sample chunks per core (out free dim)
G = 2                    # vocab rows packed per slot
S_FIELD = V // G         # 50000 slots per field
MID = S_FIELD // 2       # 25000: signed-index midpoint
ROWE = 256               # fp16 elems per slot (512 B)
NIDX = BC + TB           # 2176: padded index count (17 chunks of 128)
NCHUNK = NIDX // TB      # 17
AUXW = 19                # aux scalar vector width

_NC_CACHE = {}


def build_nc(debug=False):
    import concourse.bass as bass
    import concourse.tile as tile
    from concourse import bacc, mybir
    from contextlib import ExitStack

    f32 = mybir.dt.float32
    f16 = mybir.dt.float16
    i16 = mybir.dt.int16
    Square = mybir.ActivationFunctionType.Square
    Sigmoid = mybir.ActivationFunctionType.Sigmoid
    add = mybir.AluOpType.add

    nc = bacc.Bacc(
        "TRN2",
        target_bir_lowering=False,
        debug=debug,
        num_devices=N_CORES,
        num_swdge_queues=4,
        dynamic_dma_scratch_size=65536,
    )

    table = nc.dram_tensor("table", [F * S_FIELD, ROWE], f16, kind="ExternalInput").ap()
    idxs = nc.dram_tensor("idxs", [TB, F, NIDX // 16], i16, kind="ExternalInput").ap()
    masks = nc.dram_tensor("masks", [TB, F, NC_, G], f16, kind="ExternalInput").ap()
    dense = nc.dram_tensor("dense", [NC_, TB, N_DENSE], f32, kind="ExternalInput").ap()
    aux = nc.dram_tensor("aux", [TB, AUXW], f32, kind="ExternalInput").ap()
    fin = nc.dram_tensor("finish", [NC_, TB, 1], f32, kind="ExternalOutput").ap()
    lik = nc.dram_tensor("like", [NC_, TB, 1], f32, kind="ExternalOutput").ap()

    with tile.TileContext(nc) as tc, ExitStack() as ctx:
        ctx.enter_context(
            nc.allow_low_precision(
                reason="fp16 pair-select/field sums; 2e-2 output tolerance"
            )
        )
        singles = ctx.enter_context(tc.tile_pool(name="singles", bufs=1))
        gpool = ctx.enter_context(tc.tile_pool(name="g", bufs=6))
        wpool = ctx.enter_context(tc.tile_pool(name="w", bufs=3))
        opool = ctx.enter_context(tc.tile_pool(name="o", bufs=2))

        aux_t = singles.tile([TB, AUXW], f32)
        nc.sync.dma_start(out=aux_t[:], in_=aux[:])
        idx_t = singles.tile([TB, F, NIDX // 16], i16)
        nc.sync.dma_start(out=idx_t[:], in_=idxs[:])
        mask_t = singles.tile([TB, F, NC_, G], f16)
        nc.sync.dma_start(out=mask_t[:], in_=masks[:])
        d_t = singles.tile([TB, NC_, N_DENSE], f32)
        nc.sync.dma_start(out=d_t[:], in_=dense[:].rearrange("t p k -> p t k"))

        # accumulators over fields
        S_t = singles.tile([TB, NC_, E], f16)     # sum of selected emb2 rows
        Q_t = singles.tile([TB, NC_, E], f16)     # sum of squared selected rows
        S1_t = singles.tile([TB, NC_], f16)       # sum of selected emb1 scalars
        nc.vector.memset(S_t[:], 0.0)
        nc.vector.memset(Q_t[:], 0.0)
        nc.vector.memset(S1_t[:], 0.0)

        for f in range(F):
            g_t = gpool.tile([TB, NCHUNK, ROWE], f16)
            nc.gpsimd.dma_gather(
                g_t[:],
                table[f * S_FIELD + MID :, :],
                idx_t[:, f, :],
                NIDX,
                NIDX,
                ROWE,
                single_packet=False,
                queue_num=f % 4,
            )
            # mask-select the right row of each slot pair (g innermost)
            mb = mask_t[:, f].unsqueeze(2).broadcast_to([TB, NC_, E, G])
            m1 = wpool.tile([TB, NC_, E, G], f16)
            g2 = g_t[:, 0:NC_, 0 : E * G].rearrange("p c (e g) -> p c e g", g=G)
            nc.vector.tensor_mul(m1[:], g2, mb)
            sel = wpool.tile([TB, NC_, E], f16)
            nc.vector.tensor_reduce(
                out=sel[:], in_=m1[:], axis=mybir.AxisListType.X, op=add
            )
            nc.vector.tensor_add(S_t[:], S_t[:], sel[:])
            sq = wpool.tile([TB, NC_, E], f16)
            nc.scalar.activation(
                out=sq[:], in_=sel[:], func=Square, scale=1.0,
            )
            nc.vector.tensor_add(Q_t[:], Q_t[:], sq[:])
            # emb1 select (elems [E*G, E*G+G) of each slot)
            m2 = wpool.tile([TB, NC_, G], f16)
            nc.vector.tensor_mul(
                m2[:], g_t[:, 0:NC_, E * G : E * G + G], mask_t[:, f]
            )
            s1 = wpool.tile([TB, NC_], f16)
            nc.vector.tensor_reduce(
                out=s1[:], in_=m2[:], axis=mybir.AxisListType.X, op=add
            )
            nc.vector.tensor_add(S1_t[:], S1_t[:], s1[:])

        # ---- finalize (per sample = (partition, chunk)) ----
        # ss = sum_e S^2, qs = sum_e Q
        s2 = wpool.tile([TB, NC_, E], f32)
        nc.vector.tensor_mul(s2[:], S_t[:], S_t[:])
        ss = opool.tile([TB, NC_], f32)
        nc.vector.tensor_reduce(out=ss[:], in_=s2[:], axis=mybir.AxisListType.X, op=add)
        qs = opool.tile([TB, NC_], f32)
        nc.vector.tensor_reduce(
            out=qs[:], in_=Q_t[:], axis=mybir.AxisListType.X, op=add
        )
        # dense linear: do = sum_k dense[p,c,k] * W[k]
        wb = aux_t[:, 0:N_DENSE].unsqueeze(1).broadcast_to([TB, NC_, N_DENSE])
        dsc = wpool.tile([TB, NC_, N_DENSE], f32)
        nc.vector.tensor_mul(dsc[:], d_t[:], wb)
        do = opool.tile([TB, NC_], f32)
        nc.vector.tensor_reduce(
            out=do[:], in_=dsc[:], axis=mybir.AxisListType.X, op=add
        )
        # logits = do + b_dense + S1 + 0.5*(ss - qs)
        df = opool.tile([TB, NC_], f32)
        nc.vector.tensor_sub(df[:], ss[:], qs[:])
        l1 = opool.tile([TB, NC_], f32)
        nc.vector.tensor_scalar_mul(l1[:], df[:], 0.5)
        l2 = opool.tile([TB, NC_], f32)
        nc.vector.tensor_add(l2[:], l1[:], do[:])
        l3 = opool.tile([TB, NC_], f32)
        nc.vector.tensor_add(l3[:], l2[:], S1_t[:])
        bb = aux_t[:, 13:14].broadcast_to([TB, NC_])
        lg = opool.tile([TB, NC_], f32)
        nc.vector.tensor_add(lg[:], l3[:], bb)

        fin_t = opool.tile([TB, NC_], f32)
        lik_t = opool.tile([TB, NC_], f32)
        nc.scalar.activation(
            out=fin_t[:], in_=lg[:], func=Sigmoid,
            scale=aux_t[:, 14:15], bias=aux_t[:, 15:16],
        )
        nc.scalar.activation(
            out=lik_t[:], in_=lg[:], func=Sigmoid,
            scale=aux_t[:, 16:17], bias=aux_t[:, 17:18],
        )
        nc.sync.dma_start(
            out=fin[:].rearrange("t p e -> p t e"), in_=fin_t[:].unsqueeze(2)
        )
        nc.sync.dma_start(
            out=lik[:].rearrange("t p e -> p t e"), in_=lik_t[:].unsqueeze(2)
        )

    nc.compile()
    return nc


def _get_nc():
    if "nc" not in _NC_CACHE:
        _NC_CACHE["nc"] = build_nc(debug=False)
    return _NC_CACHE["nc"]


def _prepare_inputs(sparse_inputs, dense_inputs, emb1, emb2, W_dense, b_dense,
                    W_finish, b_finish, W_like, b_like):
    sparse_inputs = np.asarray(sparse_inputs)
    dense_inputs = np.asarray(dense_inputs, dtype=np.float32)
    emb1 = np.asarray(emb1, dtype=np.float32)
    emb2 = np.asarray(emb2, dtype=np.float32)

    # slot table: [F*S_FIELD, 256] fp16; slot s of field f covers v in
    # {2s, 2s+1}: elems[2e+g] = emb2[f, 2s+g, e]; elems[128+g] = emb1[f, 2s+g]
    T = np.zeros((F * S_FIELD, ROWE), dtype=np.float16)
    e2 = emb2.astype(np.float16).reshape(F, S_FIELD, G, E).transpose(0, 1, 3, 2)
    T[:, 0 : E * G] = e2.reshape(F * S_FIELD, E * G)
    T[:, E * G : E * G + G] = (
        emb1.astype(np.float16).reshape(F, S_FIELD, G).reshape(F * S_FIELD, G)
    )

    aux = np.zeros((TB, AUXW), dtype=np.float32)
    aux[:, 0:N_DENSE] = np.asarray(W_dense, dtype=np.float32).reshape(-1)
    aux[:, 13] = np.float32(np.asarray(b_dense).reshape(-1)[0])
    aux[:, 14] = np.float32(np.asarray(W_finish).reshape(-1)[0])
    aux[:, 15] = np.float32(np.asarray(b_finish).reshape(-1)[0])
    aux[:, 16] = np.float32(np.asarray(W_like).reshape(-1)[0])
    aux[:, 17] = np.float32(np.asarray(b_like).reshape(-1)[0])

    v_all = sparse_inputs.astype(np.int64)          # [B, F] in [0, V)
    slot_all = (v_all >> 1) - MID                   # signed slot offsets
    gsel_all = (v_all & 1).astype(np.int64)         # which row of the pair

    in_maps = []
    j = np.arange(BC)
    ch16, col = (j % 16), (j // 16)
    for c in range(N_CORES):
        sl = slice(c * BC, (c + 1) * BC)
        slot = slot_all[sl]                         # [BC, F]
        gsel = gsel_all[sl]
        # idx16 [128, F, NIDX//16]: position j -> (partition j%16 (+16k), col j//16)
        idx16 = np.full((TB, F, NIDX // 16), 1000, dtype=np.int16)
        tmp = np.zeros((16, F, NIDX // 16), dtype=np.int16)
        tmp[:, :, : BC // 16] = 1000
        tmp[ch16, :, col] = slot.astype(np.int16)
        idx16[:] = np.tile(tmp, (8, 1, 1))
        # masks [128, F, NC_, G] one-hot of gsel; sample j = chunk*128 + p
        mk = np.zeros((TB, F, NC_, G), dtype=np.float16)
        p_of_j, c_of_j = (j % TB), (j // TB)
        for g in range(G):
            sel = (gsel == g)                       # [BC, F]
            mk[p_of_j[:, None], np.arange(F)[None, :], c_of_j[:, None], g] = (
                sel.astype(np.float16)
            )
        in_maps.append(dict(
            table=T,
            idxs=idx16,
            masks=mk,
            dense=np.ascontiguousarray(
                dense_inputs[sl].reshape(NC_, TB, N_DENSE)
            ),
            aux=aux,
        ))
    return in_maps


def _install_trace_hooks():
    """Make trace=True work in containers whose antenv stub lacks axon_hooks."""
    import sys
    import types

    try:
        from antenv.axon_hooks import get_axon_ntff_profile_hook  # noqa: F401
    except ImportError:
        mod = types.ModuleType("antenv.axon_hooks")
        mod._hook = None
        mod.set_axon_ntff_profile_hook = lambda h: setattr(mod, "_hook", h)
        mod.get_axon_ntff_profile_hook = lambda: mod._hook
        sys.modules["antenv.axon_hooks"] = mod
        import antenv

        antenv.axon_hooks = mod
        from trn_agent_boot.trn_boot import _ntff_profile_via_ctypes

        mod._hook = _ntff_profile_via_ctypes("/opt/axon/libaxon_pjrt.so")

    from concourse import bass_utils

    bass_utils.upload_artifacts = lambda tmpdir: f"local://{tmpdir}"


def run(inputs, trace=False, cores=None):
    """Run on the NeuronCores; returns ((finish, like), BassKernelResults)."""
    from concourse.bass_utils import run_bass_kernel_spmd

    if trace:
        _install_trace_hooks()
    in_maps = _prepare_inputs(**inputs)
    nc = _get_nc()
    ncores = cores if cores is not None else N_CORES
    res = run_bass_kernel_spmd(nc, in_maps[:ncores], list(range(ncores)), trace=trace)
    fin = np.concatenate(
        [res.results[c]["finish"].reshape(BC, 1) for c in range(ncores)], axis=0
    )
    lik = np.concatenate(
        [res.results[c]["like"].reshape(BC, 1) for c in range(ncores)], axis=0
    )
    return (fin, lik), res


def kernel(**inputs):
    (fin, lik), _ = run(inputs, trace=bool(int(os.environ.get("KERNEL_TRACE", "0"))))
    return fin, lik


# revision 15
# speedup vs baseline: 3.0932x; 1.1612x over previous
"""FM (DeepFM first+second order) multi-task kernel for Trainium2, 8 NeuronCores.

Strategy: data-parallel over batch (2048 rows/core).  The gather uses the
dedicated SWDGE dma_gather primitive (vectorized Q7 descriptor generation,
~30x cheaper per descriptor than generic indirect DMA) with int16 indices.
To fit the 100k vocab into int16 range, two vocab rows are packed per 512-byte
table slot (slot id = v>>1 in [0, 50000)) and the call's base pointer sits at
the field's slot midpoint so signed indices (v>>1)-25000 in [-25000, 25000)
cover the whole field.  Each gathered slot holds both candidate rows
interleaved (elem[2e+g] = emb2[2s+g, e], fp16) plus both emb1 scalars; a
host-precomputed one-hot (v&1) mask selects the right candidate on-chip:
DVE mask-multiply + unit-stride pair reduce, ACT squares, tiny finalization.
26 dma_gather calls per core (one per field, 2176 idxs each incl. pad chunk
that keeps the trailing index non-negative so the ucode's trailing-trim
doesn't drop real rows).
"""

import os
import sys

import numpy as np

if "/opt/trn_rl_repo" not in sys.path:
    sys.path.insert(0, "/opt/trn_rl_repo")

N_DENSE = 13
F = 26           # n sparse fields
V = 100000       # vocab
E = 64           # emb dim
B = 16384        # global batch
N_CORES = 8
BC = B // N_CORES        # 2048 batch rows per core
TB = 128                 # SBUF partitions
NC_ = BC // TB           # 16 sample chunks per core (out free dim)
G = 2                    # vocab rows packed per slot
S_FIELD = V // G         # 50000 slots per field
MID = S_FIELD // 2       # 25000: signed-index midpoint
ROWE = 256               # fp16 elems per slot (512 B)
NIDX = BC + TB           # 2176: padded index count (17 chunks of 128)
NCHUNK = NIDX // TB      # 17
AUXW = 19                # aux scalar vector width

_NC_CACHE = {}


def build_nc(debug=False):
    import concourse.bass as bass
    import concourse.tile as tile
    from concourse import bacc, mybir
    from contextlib import ExitStack

    f32 = mybir.dt.float32
    f16 = mybir.dt.float16
    i16 = mybir.dt.int16
    Square = mybir.ActivationFunctionType.Square
    Sigmoid = mybir.ActivationFunctionType.Sigmoid
    add = mybir.AluOpType.add

    nc = bacc.Bacc(
        "TRN2",
        target_bir_lowering=False,
        debug=debug,
        num_devices=N_CORES,
        num_swdge_queues=4,
        dynamic_dma_scratch_size=65536,
    )

    table = nc.dram_tensor("table", [F * S_FIELD, ROWE], f16, kind="ExternalInput").ap()
    idxs = nc.dram_tensor("idxs", [TB, F, NIDX // 16], i16, kind="ExternalInput").ap()
    masks = nc.dram_tensor("masks", [TB, F, NC_, G], f16, kind="ExternalInput").ap()
    dense = nc.dram_tensor("dense", [TB, NC_, N_DENSE], f32, kind="ExternalInput").ap()
    aux = nc.dram_tensor("aux", [TB, AUXW], f32, kind="ExternalInput").ap()
    fin = nc.dram_tensor("finish", [TB, NC_], f32, kind="ExternalOutput").ap()
    lik = nc.dram_tensor("like", [TB, NC_], f32, kind="ExternalOutput").ap()

    with tile.TileContext(nc) as tc, ExitStack() as ctx:
        ctx.enter_context(
            nc.allow_low_precision(
                reason="fp16 pair-select/field sums; 2e-2 output tolerance"
            )
        )
        singles = ctx.enter_context(tc.tile_pool(name="singles", bufs=1))
        gpool = ctx.enter_context(tc.tile_pool(name="g", bufs=6))
        wpool = ctx.enter_context(tc.tile_pool(name="w", bufs=3))
        opool = ctx.enter_context(tc.tile_pool(name="o", bufs=2))

        # idx load first: the first dma_gather only waits on this transfer
        idx_t = singles.tile([TB, F, NIDX // 16], i16)
        nc.sync.dma_start(out=idx_t[:], in_=idxs[:])
        aux_t = singles.tile([TB, AUXW], f32)
        nc.sync.dma_start(out=aux_t[:], in_=aux[:])
        mask_t = singles.tile([TB, F, NC_, G], f16)
        nc.sync.dma_start(out=mask_t[:], in_=masks[:])
        d_t = singles.tile([TB, NC_, N_DENSE], f32)
        nc.sync.dma_start(out=d_t[:], in_=dense[:])

        # accumulators over fields
        S_t = singles.tile([TB, NC_, E], f16)     # sum of selected emb2 rows
        Q_t = singles.tile([TB, NC_, E], f16)     # sum of squared selected rows
        S1_t = singles.tile([TB, NC_], f16)       # sum of selected emb1 scalars
        nc.vector.memset(S_t[:], 0.0)
        nc.vector.memset(Q_t[:], 0.0)
        nc.vector.memset(S1_t[:], 0.0)

        for f in range(F):
            g_t = gpool.tile([TB, NCHUNK, ROWE], f16)
            nc.gpsimd.dma_gather(
                g_t[:],
                table[f * S_FIELD + MID :, :],
                idx_t[:, f, :],
                NIDX,
                NIDX,
                ROWE,
                single_packet=False,
                queue_num=f % 4,
            )
            # mask-select the right row of each slot pair (g innermost)
            mb = mask_t[:, f].unsqueeze(2).broadcast_to([TB, NC_, E, G])
            m1 = wpool.tile([TB, NC_, E, G], f16)
            g2 = g_t[:, 0:NC_, 0 : E * G].rearrange("p c (e g) -> p c e g", g=G)
            nc.vector.tensor_mul(m1[:], g2, mb)
            sel = wpool.tile([TB, NC_, E], f16)
            nc.vector.tensor_reduce(
                out=sel[:], in_=m1[:], axis=mybir.AxisListType.X, op=add
            )
            nc.vector.tensor_add(S_t[:], S_t[:], sel[:])
            sq = wpool.tile([TB, NC_, E], f16)
            nc.scalar.activation(
                out=sq[:], in_=sel[:], func=Square, scale=1.0,
            )
            nc.vector.tensor_add(Q_t[:], Q_t[:], sq[:])
            # emb1 select (elems [E*G, E*G+G) of each slot)
            m2 = wpool.tile([TB, NC_, G], f16)
            nc.vector.tensor_mul(
                m2[:], g_t[:, 0:NC_, E * G : E * G + G], mask_t[:, f]
            )
            s1 = wpool.tile([TB, NC_], f16)
            nc.vector.tensor_reduce(
                out=s1[:], in_=m2[:], axis=mybir.AxisListType.X, op=add
            )
            nc.vector.tensor_add(S1_t[:], S1_t[:], s1[:])

        # ---- finalize (per sample = (partition, chunk)) ----
        # ss = sum_e S^2, qs = sum_e Q
        s2 = wpool.tile([TB, NC_, E], f32)
        nc.vector.tensor_mul(s2[:], S_t[:], S_t[:])
        ss = opool.tile([TB, NC_], f32)
        nc.vector.tensor_reduce(out=ss[:], in_=s2[:], axis=mybir.AxisListType.X, op=add)
        qs = opool.tile([TB, NC_], f32)
        nc.vector.tensor_reduce(
            out=qs[:], in_=Q_t[:], axis=mybir.AxisListType.X, op=add
        )
        # dense linear: do = sum_k dense[p,c,k] * W[k]
        wb = aux_t[:, 0:N_DENSE].unsqueeze(1).broadcast_to([TB, NC_, N_DENSE])
        dsc = wpool.tile([TB, NC_, N_DENSE], f32)
        nc.vector.tensor_mul(dsc[:], d_t[:], wb)
        do = opool.tile([TB, NC_], f32)
        nc.vector.tensor_reduce(
            out=do[:], in_=dsc[:], axis=mybir.AxisListType.X, op=add
        )
        # logits = do + b_dense + S1 + 0.5*(ss - qs)
        df = opool.tile([TB, NC_], f32)
        nc.vector.tensor_sub(df[:], ss[:], qs[:])
        l1 = opool.tile([TB, NC_], f32)
        nc.vector.tensor_scalar_mul(l1[:], df[:], 0.5)
        l2 = opool.tile([TB, NC_], f32)
        nc.vector.tensor_add(l2[:], l1[:], do[:])
        l3 = opool.tile([TB, NC_], f32)
        nc.vector.tensor_add(l3[:], l2[:], S1_t[:])
        bb = aux_t[:, 13:14].broadcast_to([TB, NC_])
        lg = opool.tile([TB, NC_], f32)
        nc.vector.tensor_add(lg[:], l3[:], bb)

        fin_t = opool.tile([TB, NC_], f32)
        lik_t = opool.tile([TB, NC_], f32)
        nc.scalar.activation(
            out=fin_t[:], in_=lg[:], func=Sigmoid,
            scale=aux_t[:, 14:15], bias=aux_t[:, 15:16],
        )
        nc.scalar.activation(
            out=lik_t[:], in_=lg[:], func=Sigmoid,
            scale=aux_t[:, 16:17], bias=aux_t[:, 17:18],
        )
        nc.sync.dma_start(out=fin[:], in_=fin_t[:])
        nc.sync.dma_start(out=lik[:], in_=lik_t[:])

    nc.compile()
    return nc


def _get_nc():
    if "nc" not in _NC_CACHE:
        _NC_CACHE["nc"] = build_nc(debug=False)
    return _NC_CACHE["nc"]


def _prepare_inputs(sparse_inputs, dense_inputs, emb1, emb2, W_dense, b_dense,
                    W_finish, b_finish, W_like, b_like):
    sparse_inputs = np.asarray(sparse_inputs)
    dense_inputs = np.asarray(dense_inputs, dtype=np.float32)
    emb1 = np.asarray(emb1, dtype=np.float32)
    emb2 = np.asarray(emb2, dtype=np.float32)

    # slot table: [F*S_FIELD, 256] fp16; slot s of field f covers v in
    # {2s, 2s+1}: elems[2e+g] = emb2[f, 2s+g, e]; elems[128+g] = emb1[f, 2s+g]
    T = np.zeros((F * S_FIELD, ROWE), dtype=np.float16)
    e2 = emb2.astype(np.float16).reshape(F, S_FIELD, G, E).transpose(0, 1, 3, 2)
    T[:, 0 : E * G] = e2.reshape(F * S_FIELD, E * G)
    T[:, E * G : E * G + G] = (
        emb1.astype(np.float16).reshape(F, S_FIELD, G).reshape(F * S_FIELD, G)
    )

    aux = np.zeros((TB, AUXW), dtype=np.float32)
    aux[:, 0:N_DENSE] = np.asarray(W_dense, dtype=np.float32).reshape(-1)
    aux[:, 13] = np.float32(np.asarray(b_dense).reshape(-1)[0])
    aux[:, 14] = np.float32(np.asarray(W_finish).reshape(-1)[0])
    aux[:, 15] = np.float32(np.asarray(b_finish).reshape(-1)[0])
    aux[:, 16] = np.float32(np.asarray(W_like).reshape(-1)[0])
    aux[:, 17] = np.float32(np.asarray(b_like).reshape(-1)[0])

    v_all = sparse_inputs.astype(np.int64)          # [B, F] in [0, V)
    slot_all = (v_all >> 1) - MID                   # signed slot offsets
    gsel_all = (v_all & 1).astype(np.int64)         # which row of the pair

    in_maps = []
    j = np.arange(BC)
    ch16, col = (j % 16), (j // 16)
    for c in range(N_CORES):
        sl = slice(c * BC, (c + 1) * BC)
        slot = slot_all[sl]                         # [BC, F]
        gsel = gsel_all[sl]
        # idx16 [128, F, NIDX//16]: position j -> (partition j%16 (+16k), col j//16)
        idx16 = np.full((TB, F, NIDX // 16), 1000, dtype=np.int16)
        tmp = np.zeros((16, F, NIDX // 16), dtype=np.int16)
        tmp[:, :, : BC // 16] = 1000
        tmp[ch16, :, col] = slot.astype(np.int16)
        idx16[:] = np.tile(tmp, (8, 1, 1))
        # masks [128, F, NC_, G] one-hot of gsel; sample j = chunk*128 + p
        mk = np.zeros((TB, F, NC_, G), dtype=np.float16)
        p_of_j, c_of_j = (j % TB), (j // TB)
        for g in range(G):
            sel = (gsel == g)                       # [BC, F]
            mk[p_of_j[:, None], np.arange(F)[None, :], c_of_j[:, None], g] = (
                sel.astype(np.float16)
            )
        # dense pre-transposed to [TB, NC_, ND]: sample j = chunk*128 + p
        dcore = dense_inputs[sl].reshape(NC_, TB, N_DENSE)
        in_maps.append(dict(
            table=T,
            idxs=idx16,
            masks=mk,
            dense=np.ascontiguousarray(dcore.transpose(1, 0, 2)),
            aux=aux,
        ))
    return in_maps


def _install_trace_hooks():
    """Make trace=True work in containers whose antenv stub lacks axon_hooks."""
    import sys
    import types

    try:
        from antenv.axon_hooks import get_axon_ntff_profile_hook  # noqa: F401
    except ImportError:
        mod = types.ModuleType("antenv.axon_hooks")
        mod._hook = None
        mod.set_axon_ntff_profile_hook = lambda h: setattr(mod, "_hook", h)
        mod.get_axon_ntff_profile_hook = lambda: mod._hook
        sys.modules["antenv.axon_hooks"] = mod
        import antenv

        antenv.axon_hooks = mod
        from trn_agent_boot.trn_boot import _ntff_profile_via_ctypes

        mod._hook = _ntff_profile_via_ctypes("/opt/axon/libaxon_pjrt.so")

    from concourse import bass_utils

    bass_utils.upload_artifacts = lambda tmpdir: f"local://{tmpdir}"


def run(inputs, trace=False, cores=None):
    """Run on the NeuronCores; returns ((finish, like), BassKernelResults)."""
    from concourse.bass_utils import run_bass_kernel_spmd

    if trace:
        _install_trace_hooks()
    in_maps = _prepare_inputs(**inputs)
    nc = _get_nc()
    ncores = cores if cores is not None else N_CORES
    res = run_bass_kernel_spmd(nc, in_maps[:ncores], list(range(ncores)), trace=trace)
    # device layout [TB, NC_]: sample j = chunk*128 + p lives at [p, chunk]
    fin = np.concatenate(
        [
            res.results[c]["finish"].reshape(TB, NC_).T.reshape(BC, 1)
            for c in range(ncores)
        ],
        axis=0,
    )
    lik = np.concatenate(
        [
            res.results[c]["like"].reshape(TB, NC_).T.reshape(BC, 1)
            for c in range(ncores)
        ],
        axis=0,
    )
    return (fin, lik), res


def kernel(**inputs):
    (fin, lik), _ = run(inputs, trace=bool(int(os.environ.get("KERNEL_TRACE", "0"))))
    return fin, lik
